# revision 21
# baseline (speedup 1.0000x reference)
"""Trainium2 Bass kernel for nn_AttnResBlock (RMSNorm -> scalar proj ->
softmax over depth N -> weighted sum of history).

Reference computation (per (b, t) position, D=1024, N=13):
  ms      = mean_d(V^2)
  logits  = rsqrt(ms + eps) * sum_d(V * (rms_weight * w_proj))
  alpha   = softmax_n(logits)
  out     = sum_n alpha_n * V_n

Sharding: B*T = 4096 positions split contiguously across 8 cores (512
positions each). All reductions are over D and N, both core-local -> no
collectives. Tiny [D] params are replicated (pre-broadcast on host).

V is shipped bf16 (halves HBM traffic). Pos-major layout [pos, n, d] for
the weighted sum (PE diag-e matmuls) and the DVE/ACT reduction slices.

The 26 per-tile reductions over D (13 dots + 13 squares) at ~1.35us each
saturate DVE+ACT (~76us busy each at 115us total). To break that wall,
PE_N slices are ALSO shipped d-major (vdx, with the combined weight wc
appended as a 129th column). For those slices the PE computes, per
128-pos tile and 128-d chunk, the Gram block

    G = Vd^T @ [Vd | wc]   (accumulated over the 8 d-chunks in PSUM)

whose diagonal is sum_d V^2 and whose last column is the w_proj dot --
~75ns per LDW+matmul pair vs ~2.7us per slice on DVE+ACT. Extraction is
a masked TT (x identity) + tensor_reduce + a tiny dot-column copy on DVE
(~1.3us per 3 slices).

Other measured-cost-driven choices:
  - diag-e build: Pool gpsimd TT (~3ns/col) for tiles 0..2 (latency
    hidden), per-n tensor_scalar 4x split DVE/ACT for the last tile
    (end-of-kernel critical path).
  - softmax 1/sum folded into PSUM evacuation (ACT copy-with-scale /
    DVE tensor_scalar-mult), so the PE chain starts right after exp.
  - output shipped bf16 (host upcasts); saves 1MB/core of store traffic.

Engine split per tile:
  PE  : 48 Gram matmuls + 26 weighted-sum matmuls
  DVE : 7 STT dots + Gram extraction + softmax smalls + evac half1
  ACT : 7 squares (Square+accum) + ln/exp rsqrt + softmax exp + evac h0
  Pool: diag-e bank build tiles 0..2
  SP  : v loads + stores; gpsimd SWDGE: vdx/identb loads
"""

import numpy as np

import concourse.bacc as bacc
import concourse.hw_specs as hw_specs
import concourse.mybir as mybir
from concourse.bass import ts
from concourse.bass_utils import run_bass_kernel_spmd
from concourse.tile import TileContext

N, B, T, D = 13, 2, 2048, 1024
N_CORES = 8
POS_TOTAL = B * T                    # 4096
POS_PER_CORE = POS_TOTAL // N_CORES  # 512
P = 128                              # SBUF partitions = positions per tile
TILES = POS_PER_CORE // P            # 4
EPS = float(np.finfo(np.float32).eps)
CH = D // P                          # 8 d-chunks per slice

F32 = mybir.dt.float32
BF16 = mybir.dt.bfloat16
Alu = mybir.AluOpType
Act = mybir.ActivationFunctionType

K_PE = 5                         # slices whose ms+dot go through the PE Gram
PE_GROUPS = [(0, 3), (3, 5)]     # extraction groups (start, end) in PE slices
# PE slices are n = 0..K_PE-1; DVE/ACT handle n = K_PE..12
MM_FREE = 512  # free dim per matmul; PSUM bank limit for 4B

_CACHE = {}

_ACT_SET = "natural_log_exp_and_others"  # contains ln, exp, square, copy


def _patched_tables(orig):
    def fn(module_arch):
        t = orig(module_arch)
        return {k: (v if k == _ACT_SET else set()) for k, v in t.items()}

    return fn


def _build(reps=1):
    nc = bacc.Bacc(None, target_bir_lowering=False)
    v = nc.dram_tensor("v", [POS_PER_CORE, N, D], BF16, kind="ExternalInput")
    vdx = nc.dram_tensor(
        "vdx", [TILES, P, K_PE, CH, P + 1], BF16, kind="ExternalInput"
    )
    wcb = nc.dram_tensor("wcb", [P, D], BF16, kind="ExternalInput")
    o = nc.dram_tensor("o", [POS_PER_CORE, D], BF16, kind="ExternalOutput")

    with TileContext(nc) as tc:
        with (
            tc.tile_pool(name="cst", bufs=1) as cst,
            tc.tile_pool(name="vp", bufs=1) as vp,
            tc.tile_pool(name="vd", bufs=1) as vdp,
            tc.tile_pool(name="sm", bufs=3) as sm,
            tc.tile_pool(name="dg", bufs=3) as dg,
            tc.tile_pool(name="ob", bufs=2) as ob,
            tc.tile_pool(name="ps", bufs=4, space="PSUM") as psp,
            tc.tile_pool(name="ps3", bufs=4, space="PSUM") as ps3p,
        ):
            wct = cst.tile([P, D], BF16)
            idb = cst.tile([P, N, P], BF16)
            epst = cst.tile([P, 1], F32)
            trash_dve = cst.tile([P, D], BF16)
            trash_act = cst.tile([P, D], BF16)
            nc.scalar.dma_start(out=wct[:], in_=wcb[:, :])
            nc.vector.memset(epst[:], EPS)
            # identity-broadcast built on-chip (saves 0.43MB of DMA):
            # keep 1.0 where free-idx j == partition p, else 0. Demoted
            # priority so the gpsimd queue issues the vdx loads first;
            # idb isn't needed until the first Gram extraction (~25us).
            with tc.high_priority(offset=-2000):
                nc.gpsimd.memset(idb[:], 1.0)
                nc.gpsimd.affine_select(
                    out=idb[:], in_=idb[:],
                    pattern=[[0, N], [1, P]],
                    compare_op=Alu.is_equal,
                    fill=0.0, base=0, channel_multiplier=-1,
                )

            for _ in range(reps):
                # ---- global two-phase loads ----
                # All 4 vt tiles stay resident (vp bufs=4). Phase A ships
                # every tile's reduce-critical data (vdx Gram input +
                # slices K_PE..12) so DVE/ACT/PE never starve; phase B
                # ships the wsum-only slices 0..K_PE-1, which are not
                # needed until each tile's softmax is done.
                vts = [
                    vp.tile([P, N, D], BF16, name=f"vt{t}", tag=f"vt{t}")
                    for t in range(TILES)
                ]
                vdss = [
                    vdp.tile(
                        [P, K_PE, CH, P + 1], BF16,
                        name=f"vds{t}", tag=f"vds{t}",
                    )
                    for t in range(TILES)
                ]
                def phase_a(t):
                    if t == 0:
                        # fine-grained first transfers so the engines can
                        # start within ~3us of the ring opening
                        nc.gpsimd.dma_start(
                            out=vdss[t][:, 0:2, :, :], in_=vdx[t, :, 0:2, :, :]
                        )
                        nc.sync.dma_start(
                            out=vts[t][:, 5:6, :], in_=v[ts(t, P), 5:6, :]
                        )
                        nc.sync.dma_start(
                            out=vts[t][:, 6:8, :], in_=v[ts(t, P), 6:8, :]
                        )
                        nc.gpsimd.dma_start(
                            out=vdss[t][:, 2:4, :, :], in_=vdx[t, :, 2:4, :, :]
                        )
                        nc.sync.dma_start(
                            out=vts[t][:, 8:9, :], in_=v[ts(t, P), 8:9, :]
                        )
                        nc.gpsimd.dma_start(
                            out=vts[t][:, 9:11, :], in_=v[ts(t, P), 9:11, :]
                        )
                        nc.sync.dma_start(
                            out=vdss[t][:, 4:K_PE, :, :],
                            in_=vdx[t, :, 4:K_PE, :, :],
                        )
                        nc.gpsimd.dma_start(
                            out=vts[t][:, 11:13, :], in_=v[ts(t, P), 11:13, :]
                        )
                    else:
                        nc.gpsimd.dma_start(
                            out=vdss[t][:, 0:3, :, :], in_=vdx[t, :, 0:3, :, :]
                        )
                        nc.sync.dma_start(
                            out=vts[t][:, 5:9, :], in_=v[ts(t, P), 5:9, :]
                        )
                        nc.sync.dma_start(
                            out=vdss[t][:, 3:K_PE, :, :],
                            in_=vdx[t, :, 3:K_PE, :, :],
                        )
                        nc.gpsimd.dma_start(
                            out=vts[t][:, 9:13, :], in_=v[ts(t, P), 9:13, :]
                        )

                def phase_b(t):
                    nc.sync.dma_start(
                        out=vts[t][:, 0:3, :], in_=v[ts(t, P), 0:3, :]
                    )
                    nc.gpsimd.dma_start(
                        out=vts[t][:, 3:5, :], in_=v[ts(t, P), 3:5, :]
                    )

                # A0 A1 B0 A2 B1 A3 B2 B3: each tile's wsum-only slices
                # stream right behind the NEXT tile's reduce-critical data
                phase_a(0)
                phase_a(1)
                phase_b(0)
                phase_a(2)
                phase_b(1)
                phase_a(3)
                phase_b(2)
                phase_b(3)
                for t in range(TILES):
                    vt = vts[t]
                    vds = vdss[t]
                    dotv = sm.tile([P, N], F32, tag="dotv")
                    msv = sm.tile([P, N], F32, tag="msv")

                    # ---- PE Gram pass: ms+dot for slices 0..K_PE-1 ----
                    extr = sm.tile([P, K_PE, P], F32, tag="extr")
                    for gi, (nlo, nhi) in enumerate(PE_GROUPS):
                        ng = nhi - nlo
                        ps3 = ps3p.tile([P, ng, P + 4], F32, tag="ps3")
                        # pulled ahead (~1.5 tiles of instructions) so a
                        # wsum chain of an earlier tile that stalls on its
                        # phase-B slices cannot block this Gram in the PE
                        # FIFO
                        with tc.high_priority(offset=170):
                            for ni in range(ng):
                                n = nlo + ni
                                for c in range(CH):
                                    nc.tensor.matmul(
                                        ps3[:, ni, 0 : P + 1],
                                        vds[:, n, c, 0:P],
                                        vds[:, n, c, 0 : P + 1],
                                        start=(c == 0),
                                        stop=(c == CH - 1),
                                    )
                        nc.vector.tensor_tensor(
                            extr[:, nlo:nhi, :],
                            ps3[:, :, 0:P],
                            idb[:, nlo:nhi, :],
                            Alu.mult,
                        )
                        nc.vector.tensor_scalar(
                            dotv[:, nlo:nhi], ps3[:, :, P], 0.0, None,
                            Alu.bypass,
                        )
                    nc.vector.tensor_reduce(
                        out=msv[:, 0:K_PE],
                        in_=extr[:],
                        op=Alu.add,
                        axis=mybir.AxisListType.X,
                    )

                    # ---- DVE dots + ACT squares for slices K_PE..12 ----
                    for n in range(K_PE, N):
                        nc.vector.scalar_tensor_tensor(
                            out=trash_dve[:],
                            in0=vt[:, n, :],
                            scalar=0.0,
                            in1=wct[:],
                            op0=Alu.bypass,
                            op1=Alu.mult,
                            accum_out=dotv[:, n : n + 1],
                        )
                        m = K_PE + ((n - K_PE + 2) % (N - K_PE))
                        if m == K_PE:
                            # one square rides DVE to balance ACT
                            nc.vector.scalar_tensor_tensor(
                                out=trash_dve[:],
                                in0=vt[:, m, :],
                                scalar=0.0,
                                in1=vt[:, m, :],
                                op0=Alu.bypass,
                                op1=Alu.mult,
                                accum_out=msv[:, m : m + 1],
                            )
                        else:
                            nc.scalar.activation(
                                out=trash_act[:],
                                in_=vt[:, m, :],
                                func=Act.Square,
                                accum_out=msv[:, m : m + 1],
                            )

                    # ---- softmax over n (high priority: unblocks PE) ----
                    with tc.high_priority(offset=100):
                        # rsqrt(mean + eps) = exp(-0.5 * ln(ms/D + eps))
                        lnv = sm.tile([P, N], F32, tag="lnv")
                        rsq = sm.tile([P, N], F32, tag="rsq")
                        nc.scalar.activation(
                            lnv[:], msv[:], Act.Ln, bias=epst[:], scale=1.0 / D
                        )
                        nc.scalar.activation(rsq[:], lnv[:], Act.Exp, scale=-0.5)
                        lg = sm.tile([P, N], F32, tag="lg")
                        nc.vector.tensor_tensor(lg[:], dotv[:], rsq[:], Alu.mult)
                        negm = sm.tile([P, 1], F32, tag="negm")
                        nc.vector.tensor_reduce(
                            out=negm[:], in_=lg[:], op=Alu.max,
                            axis=mybir.AxisListType.X, negate=True,
                        )
                        # unnormalized exp; 1/sum rides the evacuation
                        ev = sm.tile([P, N, 1], F32, tag="ev")
                        ssum = sm.tile([P, 1], F32, tag="ssum")
                        nc.scalar.activation(
                            ev[:, :, 0], lg[:], Act.Exp, bias=negm[:],
                            accum_out=ssum[:],
                        )
                        rcp = sm.tile([P, 1], F32, tag="rcp")
                        nc.vector.reciprocal(rcp[:], ssum[:])

                        # ---- weighted sum over n: PE with diagonal-e ----
                        # diag-e build: 13 per-n tensor_scalar ops (4x on
                        # DVE) split DVE/ACT. NOT on Pool: gpsimd TT SBUF
                        # traffic was measured to slow DVE/PE ops by ~1.7x
                        # while active.
                        dgb = dg.tile([P, N, P], BF16, tag="dgb")
                        for n in range(N):
                            if n % 2 == 0:
                                nc.vector.tensor_scalar(
                                    dgb[:, n, :], idb[:, n, :],
                                    ev[:, n], None, Alu.mult,
                                )
                            else:
                                nc.scalar.mul(
                                    dgb[:, n, :], idb[:, n, :], ev[:, n]
                                )
                        osb = ob.tile([P, D], BF16, tag="osb")
                        # phase-A-resident slices first so the chain can
                        # start before the wsum-only slices (0..K_PE-1)
                        # finish streaming in
                        n_order = list(range(K_PE, N)) + list(range(K_PE))
                        for h in range(D // MM_FREE):
                            ps = psp.tile([P, MM_FREE], F32, tag="ps")
                            for i, n in enumerate(n_order):
                                nc.tensor.matmul(
                                    ps[:],
                                    dgb[:, n, :],
                                    vt[:, n, ts(h, MM_FREE)],
                                    start=(i == 0),
                                    stop=(i == N - 1),
                                )
                            # evacuation applies the softmax 1/sum; halves
                            # go to different engines so the last tile's
                            # evacuations overlap
                            if h == 0:
                                nc.scalar.mul(
                                    osb[:, ts(h, MM_FREE)], ps[:], rcp[:]
                                )
                            else:
                                nc.vector.tensor_scalar(
                                    osb[:, ts(h, MM_FREE)], ps[:],
                                    rcp[:], None, Alu.mult,
                                )
                        # one contiguous full-row store per tile (2KB
                        # per-partition runs) on the scalar HWDGE ring,
                        # which is idle at the end
                        nc.scalar.dma_start(out=o[ts(t, P), :], in_=osb[:])

    orig = hw_specs.get_activation_tables
    bacc_orig = bacc.get_activation_tables
    try:
        hw_specs.get_activation_tables = _patched_tables(orig)
        bacc.get_activation_tables = hw_specs.get_activation_tables
        nc.finalize()
    finally:
        hw_specs.get_activation_tables = orig
        bacc.get_activation_tables = bacc_orig
    return nc


def _host_prep(V, rms_weight, w_proj):
    import ml_dtypes

    bf = ml_dtypes.bfloat16
    wc32 = rms_weight.astype(np.float32) * w_proj.astype(np.float32)
    wc = wc32.astype(bf)
    wcb = np.ascontiguousarray(np.broadcast_to(wc, (P, D)))
    # [N, B*T, D] -> [B*T, N, D] so per-partition DMA runs are contiguous
    vt = np.ascontiguousarray(
        V.reshape(N, POS_TOTAL, D).transpose(1, 0, 2).astype(bf)
    )
    # d-major Gram input for the PE slices: vdx[t, d, i, c, 0:128] =
    # V[t*128+j, n=i, c*128+d]; col 128 = wc[c*128+d]. d (the SBUF
    # partition dim) comes before i so the DMA iteration orders match.
    wc_col = np.ascontiguousarray(wc.reshape(CH, P).T)  # [d, c]
    in_maps = []
    for core in range(N_CORES):
        shard = vt[core * POS_PER_CORE : (core + 1) * POS_PER_CORE]
        s5 = shard.reshape(TILES, P, N, CH, P)          # [t, j, n, c, d]
        vd = s5[:, :, :K_PE].transpose(0, 4, 2, 3, 1)   # [t, d, i, c, j]
        vdx = np.empty((TILES, P, K_PE, CH, P + 1), dtype=bf)
        vdx[..., :P] = vd
        vdx[..., P] = np.broadcast_to(
            wc_col[:, None, :], (TILES, P, K_PE, CH)
        )
        in_maps.append({"v": shard, "vdx": vdx, "wcb": wcb})
    return in_maps


def kernel(V, rms_weight, w_proj):
    if "nc" not in _CACHE:
        _CACHE["nc"] = _build()
    nc = _CACHE["nc"]
    in_maps = _host_prep(
        np.asarray(V), np.asarray(rms_weight), np.asarray(w_proj)
    )
    res = run_bass_kernel_spmd(nc, in_maps, core_ids=list(range(N_CORES)), trace=False)
    out = np.concatenate(
        [res.results[c]["o"].astype(np.float32) for c in range(N_CORES)],
        axis=0,
    )
    return out.reshape(B, T, D)


# revision 23
# speedup vs baseline: 1.0420x; 1.0420x over previous
"""Trainium2 Bass kernel for nn_AttnResBlock (RMSNorm -> scalar proj ->
softmax over depth N -> weighted sum of history).

Reference computation (per (b, t) position, D=1024, N=13):
  ms      = mean_d(V^2)
  logits  = rsqrt(ms + eps) * sum_d(V * (rms_weight * w_proj))
  alpha   = softmax_n(logits)
  out     = sum_n alpha_n * V_n

Sharding: B*T = 4096 positions split contiguously across 8 cores (512
positions each). All reductions are over D and N, both core-local -> no
collectives. Tiny [D] params are replicated (pre-broadcast on host).

V is shipped bf16 (halves HBM traffic). Pos-major layout [pos, n, d] for
the weighted sum (PE diag-e matmuls) and the DVE/ACT reduction slices.

The 26 per-tile reductions over D (13 dots + 13 squares) at ~1.35us each
saturate DVE+ACT (~76us busy each at 115us total). To break that wall,
PE_N slices are ALSO shipped d-major (vdx, with the combined weight wc
appended as a 129th column). For those slices the PE computes, per
128-pos tile and 128-d chunk, the Gram block

    G = Vd^T @ [Vd | wc]   (accumulated over the 8 d-chunks in PSUM)

whose diagonal is sum_d V^2 and whose last column is the w_proj dot --
~75ns per LDW+matmul pair vs ~2.7us per slice on DVE+ACT. Extraction is
a masked TT (x identity) + tensor_reduce + a tiny dot-column copy on DVE
(~1.3us per 3 slices).

Other measured-cost-driven choices (all HW-profiled):
  - diag-e build: 13 per-n tensor_scalar ops (4x mode on DVE) split
    DVE/ACT. NOT on Pool: gpsimd TT SBUF traffic measured to slow
    concurrent DVE/PE ops by ~1.7x. Identity built on-chip via
    affine_select (saves 0.43MB DMA).
  - softmax 1/sum folded into PSUM evacuation (ACT copy-with-scale /
    DVE tensor_scalar-mult), so the PE chain starts right after exp.
  - output shipped bf16 (host upcasts); saves 1MB/core of store traffic.
  - loads on the SP + gpsimd rings only (~310GB/s aggregate; adding the
    ACT HWDGE ring stalls ACT compute when the ring backs up); the four
    stores ride the otherwise-idle ACT ring late. Reduce-critical data
    (vdx + slices 5..12) for ALL tiles ships before the wsum-only
    slices 0..4; the wsum matmul chain runs resident slices first.

Engine split per tile:
  PE  : 40 Gram matmuls + 26 weighted-sum matmuls
  DVE : 8 STT dots + Gram extraction + softmax smalls + dgb + evac h1
  ACT : 8 squares (Square+accum) + ln/exp rsqrt + softmax exp + dgb +
        evac h0 + store issue
  SP/gpsimd: DMA rings (HWDGE / SWDGE)
"""

import numpy as np

import concourse.bacc as bacc
import concourse.hw_specs as hw_specs
import concourse.mybir as mybir
from concourse.bass import ts
from concourse.bass_utils import run_bass_kernel_spmd
from concourse.tile import TileContext

N, B, T, D = 13, 2, 2048, 1024
N_CORES = 8
POS_TOTAL = B * T                    # 4096
POS_PER_CORE = POS_TOTAL // N_CORES  # 512
P = 128                              # SBUF partitions = positions per tile
TILES = POS_PER_CORE // P            # 4
EPS = float(np.finfo(np.float32).eps)
CH = D // P                          # 8 d-chunks per slice

F32 = mybir.dt.float32
BF16 = mybir.dt.bfloat16
Alu = mybir.AluOpType
Act = mybir.ActivationFunctionType

K_PE = 5                         # slices whose ms+dot go through the PE Gram
PE_GROUPS = [(0, 3), (3, 5)]     # extraction groups (start, end) in PE slices
# PE slices are n = 0..K_PE-1; DVE/ACT handle n = K_PE..12
MM_FREE = 512  # free dim per matmul; PSUM bank limit for 4B

_CACHE = {}

_ACT_SET = "natural_log_exp_and_others"  # contains ln, exp, square, copy


def _patched_tables(orig):
    def fn(module_arch):
        t = orig(module_arch)
        return {k: (v if k == _ACT_SET else set()) for k, v in t.items()}

    return fn


def _build(reps=1):
    nc = bacc.Bacc(None, target_bir_lowering=False)
    v = nc.dram_tensor("v", [POS_PER_CORE, N, D], BF16, kind="ExternalInput")
    vdx = nc.dram_tensor(
        "vdx", [TILES, P, K_PE, CH, P + 1], BF16, kind="ExternalInput"
    )
    wcb = nc.dram_tensor("wcb", [P, D], BF16, kind="ExternalInput")
    o = nc.dram_tensor("o", [POS_PER_CORE, D], BF16, kind="ExternalOutput")

    with TileContext(nc) as tc:
        with (
            tc.tile_pool(name="cst", bufs=1) as cst,
            tc.tile_pool(name="vp", bufs=1) as vp,
            tc.tile_pool(name="vd", bufs=1) as vdp,
            tc.tile_pool(name="sm", bufs=3) as sm,
            tc.tile_pool(name="dg", bufs=3) as dg,
            tc.tile_pool(name="ob", bufs=2) as ob,
            tc.tile_pool(name="ps", bufs=4, space="PSUM") as psp,
            tc.tile_pool(name="ps3", bufs=4, space="PSUM") as ps3p,
        ):
            wct = cst.tile([P, D], BF16)
            idb = cst.tile([P, N, P], BF16)
            epst = cst.tile([P, 1], F32)
            trash_dve = cst.tile([P, D], BF16)
            trash_act = cst.tile([P, D], BF16)
            nc.scalar.dma_start(out=wct[:], in_=wcb[:, :])
            nc.vector.memset(epst[:], EPS)
            # identity-broadcast built on-chip (saves 0.43MB of DMA):
            # keep 1.0 where free-idx j == partition p, else 0. Demoted
            # priority so the gpsimd queue issues the vdx loads first;
            # idb isn't needed until the first Gram extraction (~25us).
            with tc.high_priority(offset=-2000):
                nc.gpsimd.memset(idb[:], 1.0)
                nc.gpsimd.affine_select(
                    out=idb[:], in_=idb[:],
                    pattern=[[0, N], [1, P]],
                    compare_op=Alu.is_equal,
                    fill=0.0, base=0, channel_multiplier=-1,
                )

            for _ in range(reps):
                # ---- global two-phase loads ----
                # All 4 vt tiles stay resident (vp bufs=4). Phase A ships
                # every tile's reduce-critical data (vdx Gram input +
                # slices K_PE..12) so DVE/ACT/PE never starve; phase B
                # ships the wsum-only slices 0..K_PE-1, which are not
                # needed until each tile's softmax is done.
                vts = [
                    vp.tile([P, N, D], BF16, name=f"vt{t}", tag=f"vt{t}")
                    for t in range(TILES)
                ]
                vdss = [
                    vdp.tile(
                        [P, K_PE, CH, P + 1], BF16,
                        name=f"vds{t}", tag=f"vds{t}",
                    )
                    for t in range(TILES)
                ]
                def phase_a(t):
                    if t == 0:
                        # fine-grained first transfers so the engines can
                        # start within ~3us of the ring opening
                        nc.gpsimd.dma_start(
                            out=vdss[t][:, 0:2, :, :], in_=vdx[t, :, 0:2, :, :]
                        )
                        nc.sync.dma_start(
                            out=vts[t][:, 5:6, :], in_=v[ts(t, P), 5:6, :]
                        )
                        nc.sync.dma_start(
                            out=vts[t][:, 6:8, :], in_=v[ts(t, P), 6:8, :]
                        )
                        nc.gpsimd.dma_start(
                            out=vdss[t][:, 2:4, :, :], in_=vdx[t, :, 2:4, :, :]
                        )
                        nc.sync.dma_start(
                            out=vts[t][:, 8:9, :], in_=v[ts(t, P), 8:9, :]
                        )
                        nc.gpsimd.dma_start(
                            out=vts[t][:, 9:11, :], in_=v[ts(t, P), 9:11, :]
                        )
                        nc.sync.dma_start(
                            out=vdss[t][:, 4:K_PE, :, :],
                            in_=vdx[t, :, 4:K_PE, :, :],
                        )
                        nc.gpsimd.dma_start(
                            out=vts[t][:, 11:13, :], in_=v[ts(t, P), 11:13, :]
                        )
                    else:
                        nc.gpsimd.dma_start(
                            out=vdss[t][:, 0:3, :, :], in_=vdx[t, :, 0:3, :, :]
                        )
                        nc.sync.dma_start(
                            out=vts[t][:, 5:9, :], in_=v[ts(t, P), 5:9, :]
                        )
                        nc.sync.dma_start(
                            out=vdss[t][:, 3:K_PE, :, :],
                            in_=vdx[t, :, 3:K_PE, :, :],
                        )
                        nc.gpsimd.dma_start(
                            out=vts[t][:, 9:13, :], in_=v[ts(t, P), 9:13, :]
                        )

                def phase_b(t):
                    nc.sync.dma_start(
                        out=vts[t][:, 0:3, :], in_=v[ts(t, P), 0:3, :]
                    )
                    nc.gpsimd.dma_start(
                        out=vts[t][:, 3:5, :], in_=v[ts(t, P), 3:5, :]
                    )

                # all reduce-critical data first, then the wsum-only
                # slices (v8 ordering -- measured fastest)
                for t in range(TILES):
                    phase_a(t)
                for t in range(TILES):
                    phase_b(t)
                for t in range(TILES):
                    vt = vts[t]
                    vds = vdss[t]
                    dotv = sm.tile([P, N], F32, tag="dotv")
                    msv = sm.tile([P, N], F32, tag="msv")

                    # ---- PE Gram pass: ms+dot for slices 0..K_PE-1 ----
                    extr = sm.tile([P, K_PE, P], F32, tag="extr")
                    for gi, (nlo, nhi) in enumerate(PE_GROUPS):
                        ng = nhi - nlo
                        ps3 = ps3p.tile([P, ng, P + 4], F32, tag="ps3")
                        for ni in range(ng):
                            n = nlo + ni
                            for c in range(CH):
                                nc.tensor.matmul(
                                    ps3[:, ni, 0 : P + 1],
                                    vds[:, n, c, 0:P],
                                    vds[:, n, c, 0 : P + 1],
                                    start=(c == 0),
                                    stop=(c == CH - 1),
                                )
                        nc.vector.tensor_tensor(
                            extr[:, nlo:nhi, :],
                            ps3[:, :, 0:P],
                            idb[:, nlo:nhi, :],
                            Alu.mult,
                        )
                        nc.vector.tensor_scalar(
                            dotv[:, nlo:nhi], ps3[:, :, P], 0.0, None,
                            Alu.bypass,
                        )
                    nc.vector.tensor_reduce(
                        out=msv[:, 0:K_PE],
                        in_=extr[:],
                        op=Alu.add,
                        axis=mybir.AxisListType.X,
                    )

                    # ---- DVE dots + ACT squares for slices K_PE..12 ----
                    for n in range(K_PE, N):
                        nc.vector.scalar_tensor_tensor(
                            out=trash_dve[:],
                            in0=vt[:, n, :],
                            scalar=0.0,
                            in1=wct[:],
                            op0=Alu.bypass,
                            op1=Alu.mult,
                            accum_out=dotv[:, n : n + 1],
                        )
                        m = K_PE + ((n - K_PE + 2) % (N - K_PE))
                        nc.scalar.activation(
                            out=trash_act[:],
                            in_=vt[:, m, :],
                            func=Act.Square,
                            accum_out=msv[:, m : m + 1],
                        )

                    # ---- softmax over n (high priority: unblocks PE) ----
                    with tc.high_priority(offset=100):
                        # rsqrt(mean + eps) = exp(-0.5 * ln(ms/D + eps))
                        lnv = sm.tile([P, N], F32, tag="lnv")
                        rsq = sm.tile([P, N], F32, tag="rsq")
                        nc.scalar.activation(
                            lnv[:], msv[:], Act.Ln, bias=epst[:], scale=1.0 / D
                        )
                        nc.scalar.activation(rsq[:], lnv[:], Act.Exp, scale=-0.5)
                        lg = sm.tile([P, N], F32, tag="lg")
                        nc.vector.tensor_tensor(lg[:], dotv[:], rsq[:], Alu.mult)
                        negm = sm.tile([P, 1], F32, tag="negm")
                        nc.vector.tensor_reduce(
                            out=negm[:], in_=lg[:], op=Alu.max,
                            axis=mybir.AxisListType.X, negate=True,
                        )
                        # unnormalized exp; 1/sum rides the evacuation
                        ev = sm.tile([P, N, 1], F32, tag="ev")
                        ssum = sm.tile([P, 1], F32, tag="ssum")
                        nc.scalar.activation(
                            ev[:, :, 0], lg[:], Act.Exp, bias=negm[:],
                            accum_out=ssum[:],
                        )
                        rcp = sm.tile([P, 1], F32, tag="rcp")
                        nc.vector.reciprocal(rcp[:], ssum[:])

                        # ---- weighted sum over n: PE with diagonal-e ----
                        # diag-e build: 13 per-n tensor_scalar ops (4x on
                        # DVE) split DVE/ACT. NOT on Pool: gpsimd TT SBUF
                        # traffic was measured to slow DVE/PE ops by ~1.7x
                        # while active.
                        dgb = dg.tile([P, N, P], BF16, tag="dgb")
                        for n in range(N):
                            if n % 2 == 0:
                                nc.vector.tensor_scalar(
                                    dgb[:, n, :], idb[:, n, :],
                                    ev[:, n], None, Alu.mult,
                                )
                            else:
                                nc.scalar.mul(
                                    dgb[:, n, :], idb[:, n, :], ev[:, n]
                                )
                        osb = ob.tile([P, D], BF16, tag="osb")
                        # phase-A-resident slices first so the chain can
                        # start before the wsum-only slices (0..K_PE-1)
                        # finish streaming in
                        n_order = list(range(K_PE, N)) + list(range(K_PE))
                        for h in range(D // MM_FREE):
                            ps = psp.tile([P, MM_FREE], F32, tag="ps")
                            for i, n in enumerate(n_order):
                                nc.tensor.matmul(
                                    ps[:],
                                    dgb[:, n, :],
                                    vt[:, n, ts(h, MM_FREE)],
                                    start=(i == 0),
                                    stop=(i == N - 1),
                                )
                            # evacuation applies the softmax 1/sum; halves
                            # go to different engines so the last tile's
                            # evacuations overlap
                            if h == 0:
                                nc.scalar.mul(
                                    osb[:, ts(h, MM_FREE)], ps[:], rcp[:]
                                )
                            else:
                                nc.vector.tensor_scalar(
                                    osb[:, ts(h, MM_FREE)], ps[:],
                                    rcp[:], None, Alu.mult,
                                )
                        # one contiguous full-row store per tile (2KB
                        # per-partition runs) on the scalar HWDGE ring,
                        # which is idle at the end
                        nc.scalar.dma_start(out=o[ts(t, P), :], in_=osb[:])

    orig = hw_specs.get_activation_tables
    bacc_orig = bacc.get_activation_tables
    try:
        hw_specs.get_activation_tables = _patched_tables(orig)
        bacc.get_activation_tables = hw_specs.get_activation_tables
        nc.finalize()
    finally:
        hw_specs.get_activation_tables = orig
        bacc.get_activation_tables = bacc_orig
    return nc


def _host_prep(V, rms_weight, w_proj):
    import ml_dtypes

    bf = ml_dtypes.bfloat16
    wc32 = rms_weight.astype(np.float32) * w_proj.astype(np.float32)
    wc = wc32.astype(bf)
    wcb = np.ascontiguousarray(np.broadcast_to(wc, (P, D)))
    # [N, B*T, D] -> [B*T, N, D] so per-partition DMA runs are contiguous
    vt = np.ascontiguousarray(
        V.reshape(N, POS_TOTAL, D).transpose(1, 0, 2).astype(bf)
    )
    # d-major Gram input for the PE slices: vdx[t, d, i, c, 0:128] =
    # V[t*128+j, n=i, c*128+d]; col 128 = wc[c*128+d]. d (the SBUF
    # partition dim) comes before i so the DMA iteration orders match.
    wc_col = np.ascontiguousarray(wc.reshape(CH, P).T)  # [d, c]
    in_maps = []
    for core in range(N_CORES):
        shard = vt[core * POS_PER_CORE : (core + 1) * POS_PER_CORE]
        s5 = shard.reshape(TILES, P, N, CH, P)          # [t, j, n, c, d]
        vd = s5[:, :, :K_PE].transpose(0, 4, 2, 3, 1)   # [t, d, i, c, j]
        vdx = np.empty((TILES, P, K_PE, CH, P + 1), dtype=bf)
        vdx[..., :P] = vd
        vdx[..., P] = np.broadcast_to(
            wc_col[:, None, :], (TILES, P, K_PE, CH)
        )
        in_maps.append({"v": shard, "vdx": vdx, "wcb": wcb})
    return in_maps


def kernel(V, rms_weight, w_proj):
    if "nc" not in _CACHE:
        _CACHE["nc"] = _build()
    nc = _CACHE["nc"]
    in_maps = _host_prep(
        np.asarray(V), np.asarray(rms_weight), np.asarray(w_proj)
    )
    res = run_bass_kernel_spmd(nc, in_maps, core_ids=list(range(N_CORES)), trace=False)
    out = np.concatenate(
        [res.results[c]["o"].astype(np.float32) for c in range(N_CORES)],
        axis=0,
    )
    return out.reshape(B, T, D)
